# revision 6
# baseline (speedup 1.0000x reference)
"""Trainium2 Bass kernel for nn_KernelQM9 (e3nn-style tensor-product kernel).

Contract: kernel(**inputs) takes the FULL inputs
    r  [16,32,32,3] f32,  W1 [1,64],  b1 [64],  W2 [64,1536],  b2 [1536]
and returns the FULL output [16,32,32,64,64] f32.

Strategy: data-parallel over the flattened point index z = (batch,A,B)
(16384 points) across 8 NeuronCores (2048 points each).  Per core:
  - radial MLP R = silu(|r|*W1+b1) @ W2aug on the PE (bias via ones-row),
    with W2 columns pre-permuted on the host so every consumer slice of the
    result V is contiguous,
  - real spherical harmonics built with a handful of small vector ops,
  - the 64x64 block kernel assembled with per-partition-scalar multiplies
    (tensor_scalar / scalar_tensor_tensor / activation-scale) split across
    the Vector and Scalar engines,
  - one 2MB DMA store per 128-point tile.
The problem is HBM-write-bound (256MB output): ~32MB/core at ~360GB/s.
"""
import math
import os

import numpy as np

# ---------------------------------------------------------------------------
# constants derived from the e3nn structure (numpy only, self-contained)
# ---------------------------------------------------------------------------


def _f(n):
    return math.factorial(round(n))


def _su2_cg(j1, m1, j2, m2, j3, m3):
    if m3 != m1 + m2:
        return 0.0
    vmin = max(-j1 + j2 + m3, -j1 + m1, 0)
    vmax = min(j2 + j3 + m1, j3 - j1 + j2, j3 + m3)
    c = ((2 * j3 + 1) * _f(j3 + j1 - j2) * _f(j3 - j1 + j2) * _f(j1 + j2 - j3)
         / _f(j1 + j2 + j3 + 1)) ** 0.5 \
        * (_f(j3 + m3) * _f(j3 - m3)
           / (_f(j1 + m1) * _f(j1 - m1) * _f(j2 + m2) * _f(j2 - m2))) ** 0.5
    s = 0.0
    for v in range(vmin, vmax + 1):
        s += (-1.0) ** (v + j2 + m2) * _f(j2 + j3 + m1 - v) * _f(j1 - m1 + v) / (
            _f(v) * _f(j3 - j1 + j2 - v) * _f(j3 + m3 - v) * _f(v + j1 - j2 - m3))
    return c * s


def _w3j(l1, l2, l3):
    w = np.zeros((2 * l1 + 1, 2 * l2 + 1, 2 * l3 + 1))
    for m1 in range(-l1, l1 + 1):
        for m2 in range(-l2, l2 + 1):
            m3 = -(m1 + m2)
            if abs(m3) > l3:
                continue
            w[m1 + l1, m2 + l2, m3 + l3] = (-1.0) ** (l1 - l2 + m1 + m2) \
                / (2 * l3 + 1) ** 0.5 * _su2_cg(l1, m1, l2, m2, l3, -m3)
    return w


def _Q(l):
    q = np.zeros((2 * l + 1, 2 * l + 1), dtype=complex)
    q[l, l] = 1.0
    s2 = 2.0 ** -0.5
    for m in range(1, l + 1):
        q[l + m, l - m] = s2
        q[l + m, l + m] = (-1) ** m * s2
        q[l - m, l - m] = 1j * s2
        q[l - m, l + m] = -1j * (-1) ** m * s2
    return q


def _real3j(l1, l2, l3):
    C = np.einsum('ai,bj,ck,ijk->abc', _Q(l1), _Q(l2), _Q(l3),
                  _w3j(l1, l2, l3).astype(complex))
    k = int(np.argmax(np.abs(C)))
    ph = C.flat[k]
    if abs(ph) < 1e-12:
        return np.real(C)
    return np.real(C / (ph / abs(ph)))


def _derive_consts():
    RS = [(16, 0), (16, 1)]

    def _lfr(li, lo):
        return list(range(abs(li - lo), li + lo + 1))

    NORM = np.zeros((2, 2, 2))
    for i, (mo, lo) in enumerate(RS):
        nse = sum(mi * len(_lfr(li, lo)) for mi, li in RS)
        for j, (mi, li) in enumerate(RS):
            lm = math.sqrt(2 * li + 1) * math.sqrt(4 * math.pi)
            NORM[i, j, 0] = lm / math.sqrt(nse)
            NORM[i, j, 1] = lm / math.sqrt(mi)

    CG000 = _real3j(0, 0, 0)
    CG011 = _real3j(0, 1, 1)
    CG101 = _real3j(1, 0, 1)
    CG110 = _real3j(1, 1, 0)
    CG111 = _real3j(1, 1, 1)
    CG112 = _real3j(1, 1, 2)

    tol = 1e-10
    d011 = CG011[0, 0, 0]
    assert np.allclose(CG011[0], d011 * np.eye(3), atol=tol)
    d101 = CG101[0, 0, 0]
    assert np.allclose(CG101[:, 0, :], d101 * np.eye(3), atol=tol)
    d110 = CG110[0, 0, 0]
    assert np.allclose(CG110[:, :, 0], d110 * np.eye(3), atol=tol)
    w1 = CG111[0, 1, 2]
    expect111 = np.zeros((3, 3, 3))
    for (a, b, mf, s) in [(0, 1, 2, 1), (0, 2, 1, -1), (1, 2, 0, 1)]:
        expect111[a, b, mf] = s * w1
        expect111[b, a, mf] = -s * w1
    assert np.allclose(CG111, expect111, atol=tol)
    w2c = CG112[0, 1, 1]
    g6 = CG112[0, 0, 2]
    g8 = CG112[2, 2, 4]
    expect112 = np.zeros((3, 3, 5))
    for (a, b, mf) in [(0, 1, 1), (0, 2, 0), (1, 2, 3)]:
        expect112[a, b, mf] = w2c
        expect112[b, a, mf] = w2c
    expect112[0, 0, 2] = g6
    expect112[1, 1, 2] = -2 * g6
    expect112[2, 2, 2] = g6
    expect112[0, 0, 4] = -g8
    expect112[2, 2, 4] = g8
    assert np.allclose(CG112, expect112, atol=tol)

    PI = math.pi
    c0 = 0.5 / math.sqrt(PI)
    c1 = math.sqrt(3.0 / (4 * PI))
    c2 = math.sqrt(15.0 / (4 * PI))
    c20 = math.sqrt(5.0 / (16 * PI))
    snc00, snc01 = NORM[0, 0, 0], NORM[0, 1, 0]
    snc10, snc11 = NORM[1, 0, 0], NORM[1, 1, 0]
    return dict(
        a00=float(CG000[0, 0, 0] * c0 * snc00),
        q01=float(d011 * c1 * snc01),
        q10=float(d101 * c1 * snc10),
        w1p=float(w1 * c1 * snc11),
        w0c0=float(d110 * c0 * snc11),
        k2=float(w2c * c2 * snc11),
        k6=float(g6 * c20 * snc11),
        k8=float(g8 * 0.5 * c2 * snc11),
    )


_C = _derive_consts()

N_CORES = 8
NZ = 2048            # points per core
NT = 16              # tiles per core
P = 128              # points per tile (partition dim)
# sigma: l=1 m-index -> cartesian component (Y[1]=c1*y, Y[2]=c1*z, Y[3]=c1*x)
SIGMA = [1, 2, 0]
PAIRS = [(0, 1), (0, 2), (1, 2)]
PAIR_L1_COMP = [0, 2, 1]
PAIR_L1_SIGN = [1, -1, 1]


def _w2_perm():
    return np.concatenate([
        np.arange(768),
        768 + 3 * np.arange(256) + 0,
        768 + 3 * np.arange(256) + 1,
        768 + 3 * np.arange(256) + 2,
    ])


# ---------------------------------------------------------------------------
# Bass program (one core; SPMD across 8)
# ---------------------------------------------------------------------------

_NC_CACHE = {}


def _build_nc():
    if "nc" in _NC_CACHE:
        return _NC_CACHE["nc"]
    import concourse.bacc as bacc
    import concourse.bass as bass
    import concourse.tile as tile
    from concourse import mybir
    from concourse.masks import make_identity

    AF = mybir.ActivationFunctionType
    OP = mybir.AluOpType
    f32 = mybir.dt.float32

    nc = bacc.Bacc("TRN2", target_bir_lowering=False, debug=False,
                   num_devices=N_CORES)

    r_in = nc.dram_tensor("r_in", [NZ, 3], f32, kind="ExternalInput")
    w2m_in = nc.dram_tensor("w2m", [65, 1536], f32, kind="ExternalInput")
    w1_in = nc.dram_tensor("w1c", [64, 1], f32, kind="ExternalInput")
    b1_in = nc.dram_tensor("b1c", [64, 1], f32, kind="ExternalInput")
    out = nc.dram_tensor("out", [NZ, 4096], f32, kind="ExternalOutput")

    with tile.TileContext(nc) as tc:
        with (
            tc.tile_pool(name="consts", bufs=1) as consts,
            tc.tile_pool(name="prep", bufs=1) as prep,
            tc.tile_pool(name="ht", bufs=3) as htp,
            tc.tile_pool(name="vsb", bufs=3) as vsb,
            tc.tile_pool(name="ktp", bufs=3) as ktp,
            tc.tile_pool(name="tmps", bufs=4) as tmps,
            tc.tile_pool(name="vpsum", bufs=2, space="PSUM") as vpsum,
            tc.tile_pool(name="ppsum", bufs=1, space="PSUM") as ppsum,
        ):
            # ---- constants ----
            w2m = consts.tile([65, 1536], f32)
            nc.sync.dma_start(w2m, w2m_in[:, :])
            w1t = consts.tile([64, 1], f32)
            nc.sync.dma_start(w1t, w1_in[:, :])
            b1t = consts.tile([64, 1], f32)
            nc.sync.dma_start(b1t, b1_in[:, :])
            ident = consts.tile([128, 128], f32)
            make_identity(nc, ident)

            # ---- load r, batched prep over all 16 tiles ----
            # r3[p, t, c] = r[16p + t, c]  (z = 16p + t, p-major)
            r3 = prep.tile([P, NT, 3], f32)
            nc.sync.dma_start(r3, r_in.ap().rearrange("(p t) c -> p t c", p=P))

            rsq = prep.tile([P, NT, 3], f32)
            nc.vector.tensor_tensor(rsq, r3, r3, op=OP.mult)
            r2 = prep.tile([P, NT], f32)
            nc.vector.tensor_reduce(r2, rsq, axis=mybir.AxisListType.X,
                                    op=OP.add)
            radii = prep.tile([P, NT], f32)
            nc.scalar.activation(radii, r2, AF.Sqrt)
            inv2 = prep.tile([P, NT], f32)
            nc.vector.reciprocal(inv2, r2)
            inv = prep.tile([P, NT], f32)
            nc.scalar.activation(inv, inv2, AF.Sqrt)

            # t1 = r * inv ; then 3 prescaled copies
            t1 = prep.tile([P, NT, 3], f32)
            for c in range(3):
                nc.vector.tensor_tensor(t1[:, :, c], r3[:, :, c], inv,
                                        op=OP.mult)
            t1a = prep.tile([P, NT, 3], f32)
            nc.vector.tensor_scalar(t1a, t1, _C["q01"], None, op0=OP.mult)
            t1b = prep.tile([P, NT, 3], f32)
            nc.vector.tensor_scalar(t1b, t1, _C["q10"], None, op0=OP.mult)
            t1c = prep.tile([P, NT, 3], f32)
            nc.vector.tensor_scalar(t1c, t1, _C["w1p"], None, op0=OP.mult)

            # praw = k2 * (y*z, x*y, x*z);  t2w = praw * inv2
            praw = prep.tile([P, NT, 3], f32)
            nc.vector.scalar_tensor_tensor(
                praw[:, :, 0], r3[:, :, 1], _C["k2"], r3[:, :, 2],
                op0=OP.mult, op1=OP.mult)
            nc.vector.scalar_tensor_tensor(
                praw[:, :, 1], r3[:, :, 0], _C["k2"], r3[:, :, 1],
                op0=OP.mult, op1=OP.mult)
            nc.vector.scalar_tensor_tensor(
                praw[:, :, 2], r3[:, :, 0], _C["k2"], r3[:, :, 2],
                op0=OP.mult, op1=OP.mult)
            t2w = prep.tile([P, NT, 3], f32)
            for c in range(3):
                nc.vector.tensor_tensor(t2w[:, :, c], praw[:, :, c], inv2,
                                        op=OP.mult)

            # A2d combos
            qz2 = prep.tile([P, NT], f32)
            nc.vector.scalar_tensor_tensor(qz2, rsq[:, :, 2], 3.0, r2,
                                           op0=OP.mult, op1=OP.subtract)
            y6a = prep.tile([P, NT], f32)
            nc.vector.scalar_tensor_tensor(y6a, qz2, _C["k6"], inv2,
                                           op0=OP.mult, op1=OP.mult)
            qxmy = prep.tile([P, NT], f32)
            nc.vector.tensor_tensor(qxmy, rsq[:, :, 0], rsq[:, :, 1],
                                    op=OP.subtract)
            y8b = prep.tile([P, NT], f32)
            nc.vector.scalar_tensor_tensor(y8b, qxmy, _C["k8"], inv2,
                                           op0=OP.mult, op1=OP.mult)
            a2d = prep.tile([P, NT, 3], f32)
            nc.vector.tensor_tensor(a2d[:, :, 0], y6a, y8b, op=OP.subtract)
            nc.vector.tensor_scalar(a2d[:, :, 1], y6a, -2.0, None, op0=OP.mult)
            nc.vector.tensor_tensor(a2d[:, :, 2], y6a, y8b, op=OP.add)

            # W1/b1 broadcast to all partitions (z-layout MLP input)
            w1b = consts.tile([P, 64], f32)
            nc.sync.dma_start(
                w1b, bass.AP(tensor=w1_in, offset=0, ap=[[0, P], [1, 64]]))
            b1b = consts.tile([P, 64], f32)
            nc.sync.dma_start(
                b1b, bass.AP(tensor=b1_in, offset=0, ap=[[0, P], [1, 64]]))

            # ---- per-tile loop ----
            for t in range(NT):
                # h[z, f] = silu(radii[z]*W1[f] + b1[f]) for z = 16p + t
                hpre = tmps.tile([P, 64], f32, tag="hpre")
                nc.vector.scalar_tensor_tensor(
                    hpre, w1b, radii[:, t:t + 1], b1b,
                    op0=OP.mult, op1=OP.add)
                h = tmps.tile([P, 64], f32, tag="h")
                nc.scalar.activation(h, hpre, AF.Silu)
                hT_ps = ppsum.tile([64, P], f32, tag="hT_ps")
                nc.tensor.transpose(hT_ps, h, ident)
                hT = htp.tile([65, P], f32)
                nc.scalar.copy(hT[0:64, :], hT_ps)
                nc.vector.memset(hT[64:65, :], 1.0)

                # V = hTaug.T @ W2m   [128, 1536] in PSUM
                V = vpsum.tile([P, 1536], f32, tag="V")
                for c in range(3):
                    nc.tensor.matmul(V[:, c * 512:(c + 1) * 512], hT,
                                     w2m[:, c * 512:(c + 1) * 512])

                kt = ktp.tile([P, 4096], f32)
                ktv = kt.rearrange("p (a b) -> p a b", a=64)

                # --- ScalarE ops (read V straight from PSUM) ---
                vl2 = vsb.tile([P, 256], f32)
                nc.scalar.copy(vl2, V[:, 1280:1536])
                v01 = V[:, 256:512].rearrange("p (u v) -> p u v", u=16)
                for jm in range(3):
                    nc.scalar.mul(
                        ktv[:, 0:16, 16 + jm::3], v01,
                        t1a[:, t, SIGMA[jm]:SIGMA[jm] + 1])
                v10 = V[:, 512:768].rearrange("p (u v) -> p u v", u=16)
                for im in range(3):
                    nc.scalar.mul(
                        ktv[:, 16 + im::3, 0:16], v10,
                        t1b[:, t, SIGMA[im]:SIGMA[im] + 1])
                tmpd = []
                v11_1 = V[:, 1024:1280]
                for pi in range(3):
                    td = tmps.tile([P, 256], f32, tag=f"td{pi}")
                    comp = PAIR_L1_COMP[pi]
                    nc.scalar.mul(td, v11_1, t1c[:, t, comp:comp + 1])
                    tmpd.append(td)

                # --- VectorE ops ---
                nc.vector.tensor_scalar(
                    ktv[:, 0:16, 0:16],
                    V[:, 0:256].rearrange("p (u v) -> p u v", u=16),
                    _C["a00"], None, op0=OP.mult)
                vl0s = vsb.tile([P, 256], f32, tag="vl0s")
                nc.vector.tensor_scalar(vl0s, V[:, 768:1024], _C["w0c0"],
                                        None, op0=OP.mult)
                vl2v = vl2.rearrange("p (u v) -> p u v", u=16)
                vl0sv = vl0s.rearrange("p (u v) -> p u v", u=16)
                for i in range(3):
                    nc.vector.scalar_tensor_tensor(
                        ktv[:, 16 + i::3, 16 + i::3], vl2v,
                        a2d[:, t, i:i + 1], vl0sv,
                        op0=OP.mult, op1=OP.add)
                for pi, (a, b) in enumerate(PAIRS):
                    tdv = tmpd[pi].rearrange("p (u v) -> p u v", u=16)
                    s = PAIR_L1_SIGN[pi]
                    op_ab = OP.add if s > 0 else OP.subtract
                    op_ba = OP.subtract if s > 0 else OP.add
                    nc.vector.scalar_tensor_tensor(
                        ktv[:, 16 + a::3, 16 + b::3], vl2v,
                        t2w[:, t, pi:pi + 1], tdv,
                        op0=OP.mult, op1=op_ab)
                    nc.vector.scalar_tensor_tensor(
                        ktv[:, 16 + b::3, 16 + a::3], vl2v,
                        t2w[:, t, pi:pi + 1], tdv,
                        op0=OP.mult, op1=op_ba)

                # store: rows z = 16p + t
                out_ap = bass.AP(
                    tensor=out, offset=t * 4096,
                    ap=[[16 * 4096, P], [1, 4096]])
                nc.sync.dma_start(out_ap, kt)

    nc.compile()
    _NC_CACHE["nc"] = nc
    return nc


# ---------------------------------------------------------------------------
# host entry
# ---------------------------------------------------------------------------


def _run(inputs, trace=False):
    from concourse.bass_utils import run_bass_kernel_spmd

    r = np.ascontiguousarray(np.asarray(inputs["r"], np.float32).reshape(-1, 3))
    W1 = np.asarray(inputs["W1"], np.float32)
    b1 = np.asarray(inputs["b1"], np.float32)
    W2 = np.asarray(inputs["W2"], np.float32)
    b2 = np.asarray(inputs["b2"], np.float32)

    w2aug = np.concatenate([W2, b2[None, :]], 0)
    w2m = np.ascontiguousarray(w2aug[:, _w2_perm()], np.float32)
    w1c = np.ascontiguousarray(W1.reshape(64, 1))
    b1c = np.ascontiguousarray(b1.reshape(64, 1))

    nc = _build_nc()
    in_maps = []
    for c in range(N_CORES):
        in_maps.append({
            "r_in": np.ascontiguousarray(r[c * NZ:(c + 1) * NZ]),
            "w2m": w2m,
            "w1c": w1c,
            "b1c": b1c,
        })
    res = run_bass_kernel_spmd(nc, in_maps, core_ids=list(range(N_CORES)),
                               trace=trace)
    outs = [res.results[c]["out"] for c in range(N_CORES)]
    full = np.concatenate(outs, 0).reshape(16, 32, 32, 64, 64)
    return full, res


def kernel(**inputs):
    return _run(inputs, trace=False)[0]


if __name__ == "__main__":
    _build_nc()
    print("build+compile OK")


# revision 16
# speedup vs baseline: 1.1811x; 1.1811x over previous
"""Trainium2 Bass kernel for nn_KernelQM9 (e3nn-style tensor-product kernel).

Contract: kernel(**inputs) takes the FULL inputs
    r  [16,32,32,3] f32,  W1 [1,64],  b1 [64],  W2 [64,1536],  b2 [1536]
and returns the FULL output [16,32,32,64,64] f32.

Strategy: data-parallel over the flattened point index z = (batch,A,B)
(16384 points) across 8 NeuronCores (2048 points each).  Per core:
  - radial MLP R = silu(|r|*W1+b1) @ W2aug on the PE (bias via ones-row),
    with W2 columns pre-permuted on the host so every consumer slice of the
    result V is contiguous,
  - real spherical harmonics built with a handful of small vector ops,
  - the 64x64 block kernel assembled with per-partition-scalar multiplies
    (tensor_scalar / scalar_tensor_tensor / activation-scale) split across
    the Vector and Scalar engines,
  - one 2MB DMA store per 128-point tile.
The problem is HBM-write-bound (256MB output): ~32MB/core at ~360GB/s.
"""
import math
import os

import numpy as np

# ---------------------------------------------------------------------------
# constants derived from the e3nn structure (numpy only, self-contained)
# ---------------------------------------------------------------------------


def _f(n):
    return math.factorial(round(n))


def _su2_cg(j1, m1, j2, m2, j3, m3):
    if m3 != m1 + m2:
        return 0.0
    vmin = max(-j1 + j2 + m3, -j1 + m1, 0)
    vmax = min(j2 + j3 + m1, j3 - j1 + j2, j3 + m3)
    c = ((2 * j3 + 1) * _f(j3 + j1 - j2) * _f(j3 - j1 + j2) * _f(j1 + j2 - j3)
         / _f(j1 + j2 + j3 + 1)) ** 0.5 \
        * (_f(j3 + m3) * _f(j3 - m3)
           / (_f(j1 + m1) * _f(j1 - m1) * _f(j2 + m2) * _f(j2 - m2))) ** 0.5
    s = 0.0
    for v in range(vmin, vmax + 1):
        s += (-1.0) ** (v + j2 + m2) * _f(j2 + j3 + m1 - v) * _f(j1 - m1 + v) / (
            _f(v) * _f(j3 - j1 + j2 - v) * _f(j3 + m3 - v) * _f(v + j1 - j2 - m3))
    return c * s


def _w3j(l1, l2, l3):
    w = np.zeros((2 * l1 + 1, 2 * l2 + 1, 2 * l3 + 1))
    for m1 in range(-l1, l1 + 1):
        for m2 in range(-l2, l2 + 1):
            m3 = -(m1 + m2)
            if abs(m3) > l3:
                continue
            w[m1 + l1, m2 + l2, m3 + l3] = (-1.0) ** (l1 - l2 + m1 + m2) \
                / (2 * l3 + 1) ** 0.5 * _su2_cg(l1, m1, l2, m2, l3, -m3)
    return w


def _Q(l):
    q = np.zeros((2 * l + 1, 2 * l + 1), dtype=complex)
    q[l, l] = 1.0
    s2 = 2.0 ** -0.5
    for m in range(1, l + 1):
        q[l + m, l - m] = s2
        q[l + m, l + m] = (-1) ** m * s2
        q[l - m, l - m] = 1j * s2
        q[l - m, l + m] = -1j * (-1) ** m * s2
    return q


def _real3j(l1, l2, l3):
    C = np.einsum('ai,bj,ck,ijk->abc', _Q(l1), _Q(l2), _Q(l3),
                  _w3j(l1, l2, l3).astype(complex))
    k = int(np.argmax(np.abs(C)))
    ph = C.flat[k]
    if abs(ph) < 1e-12:
        return np.real(C)
    return np.real(C / (ph / abs(ph)))


def _derive_consts():
    RS = [(16, 0), (16, 1)]

    def _lfr(li, lo):
        return list(range(abs(li - lo), li + lo + 1))

    NORM = np.zeros((2, 2, 2))
    for i, (mo, lo) in enumerate(RS):
        nse = sum(mi * len(_lfr(li, lo)) for mi, li in RS)
        for j, (mi, li) in enumerate(RS):
            lm = math.sqrt(2 * li + 1) * math.sqrt(4 * math.pi)
            NORM[i, j, 0] = lm / math.sqrt(nse)
            NORM[i, j, 1] = lm / math.sqrt(mi)

    CG000 = _real3j(0, 0, 0)
    CG011 = _real3j(0, 1, 1)
    CG101 = _real3j(1, 0, 1)
    CG110 = _real3j(1, 1, 0)
    CG111 = _real3j(1, 1, 1)
    CG112 = _real3j(1, 1, 2)

    tol = 1e-10
    d011 = CG011[0, 0, 0]
    assert np.allclose(CG011[0], d011 * np.eye(3), atol=tol)
    d101 = CG101[0, 0, 0]
    assert np.allclose(CG101[:, 0, :], d101 * np.eye(3), atol=tol)
    d110 = CG110[0, 0, 0]
    assert np.allclose(CG110[:, :, 0], d110 * np.eye(3), atol=tol)
    w1 = CG111[0, 1, 2]
    expect111 = np.zeros((3, 3, 3))
    for (a, b, mf, s) in [(0, 1, 2, 1), (0, 2, 1, -1), (1, 2, 0, 1)]:
        expect111[a, b, mf] = s * w1
        expect111[b, a, mf] = -s * w1
    assert np.allclose(CG111, expect111, atol=tol)
    w2c = CG112[0, 1, 1]
    g6 = CG112[0, 0, 2]
    g8 = CG112[2, 2, 4]
    expect112 = np.zeros((3, 3, 5))
    for (a, b, mf) in [(0, 1, 1), (0, 2, 0), (1, 2, 3)]:
        expect112[a, b, mf] = w2c
        expect112[b, a, mf] = w2c
    expect112[0, 0, 2] = g6
    expect112[1, 1, 2] = -2 * g6
    expect112[2, 2, 2] = g6
    expect112[0, 0, 4] = -g8
    expect112[2, 2, 4] = g8
    assert np.allclose(CG112, expect112, atol=tol)

    PI = math.pi
    c0 = 0.5 / math.sqrt(PI)
    c1 = math.sqrt(3.0 / (4 * PI))
    c2 = math.sqrt(15.0 / (4 * PI))
    c20 = math.sqrt(5.0 / (16 * PI))
    snc00, snc01 = NORM[0, 0, 0], NORM[0, 1, 0]
    snc10, snc11 = NORM[1, 0, 0], NORM[1, 1, 0]
    return dict(
        a00=float(CG000[0, 0, 0] * c0 * snc00),
        q01=float(d011 * c1 * snc01),
        q10=float(d101 * c1 * snc10),
        w1p=float(w1 * c1 * snc11),
        w0c0=float(d110 * c0 * snc11),
        k2=float(w2c * c2 * snc11),
        k6=float(g6 * c20 * snc11),
        k8=float(g8 * 0.5 * c2 * snc11),
    )


_C = _derive_consts()

N_CORES = 8
NZ = 2048            # points per core
NT = 16              # tiles per core
P = 128              # points per tile (partition dim)
# sigma: l=1 m-index -> cartesian component (Y[1]=c1*y, Y[2]=c1*z, Y[3]=c1*x)
SIGMA = [1, 2, 0]
PAIRS = [(0, 1), (0, 2), (1, 2)]
PAIR_L1_COMP = [0, 2, 1]
PAIR_L1_SIGN = [1, -1, 1]


def _w2_perm():
    return np.concatenate([
        np.arange(768),
        768 + 3 * np.arange(256) + 0,
        768 + 3 * np.arange(256) + 1,
        768 + 3 * np.arange(256) + 2,
    ])


# ---------------------------------------------------------------------------
# Bass program (one core; SPMD across 8)
# ---------------------------------------------------------------------------

_NC_CACHE = {}


def _build_nc():
    if "nc" in _NC_CACHE:
        return _NC_CACHE["nc"]
    import concourse.bacc as bacc
    import concourse.bass as bass
    import concourse.tile as tile
    from concourse import mybir
    from concourse.masks import make_identity

    AF = mybir.ActivationFunctionType
    OP = mybir.AluOpType
    f32 = mybir.dt.float32

    nc = bacc.Bacc("TRN2", target_bir_lowering=False, debug=False,
                   num_devices=N_CORES)

    f32r = mybir.dt.float32r
    r_in = nc.dram_tensor("r_in", [NZ, 3], f32, kind="ExternalInput")
    w2m_in = nc.dram_tensor("w2m", [65, 1536], f32r, kind="ExternalInput")
    w1_in = nc.dram_tensor("w1c", [64, 1], f32, kind="ExternalInput")
    b1_in = nc.dram_tensor("b1c", [64, 1], f32, kind="ExternalInput")
    out = nc.dram_tensor("out", [NZ, 4096], f32, kind="ExternalOutput")

    with tile.TileContext(nc) as tc:
        with (
            tc.tile_pool(name="consts", bufs=1) as consts,
            tc.tile_pool(name="prep", bufs=1) as prep,
            tc.tile_pool(name="ht", bufs=3) as htp,
            tc.tile_pool(name="vsb", bufs=3) as vsb,
            tc.tile_pool(name="ktp", bufs=3) as ktp,
            tc.tile_pool(name="tmps", bufs=4) as tmps,
            tc.tile_pool(name="vpsum", bufs=2, space="PSUM") as vpsum,
            tc.tile_pool(name="ppsum", bufs=1, space="PSUM") as ppsum,
        ):
            # ---- constants ----
            w2m = consts.tile([65, 1536], f32r)
            nc.sync.dma_start(w2m, w2m_in[:, :])
            ones_row = consts.tile([1, P], f32)
            nc.vector.memset(ones_row, 1.0)
            ident = consts.tile([128, 128], f32)
            make_identity(nc, ident)

            # ---- load r, batched prep over all 16 tiles ----
            # r3[p, t, c] = r[16p + t, c]  (z = 16p + t, p-major)
            r3 = prep.tile([P, NT, 3], f32)
            nc.sync.dma_start(r3, r_in.ap().rearrange("(p t) c -> p t c", p=P))

            rsq = prep.tile([P, NT, 3], f32)
            nc.vector.tensor_tensor(rsq, r3, r3, op=OP.mult)
            r2 = prep.tile([P, NT], f32)
            nc.vector.tensor_reduce(r2, rsq, axis=mybir.AxisListType.X,
                                    op=OP.add)
            radii = prep.tile([P, NT], f32)
            nc.scalar.activation(radii, r2, AF.Sqrt)
            inv2 = prep.tile([P, NT], f32)
            nc.vector.reciprocal(inv2, r2)
            inv = prep.tile([P, NT], f32)
            nc.scalar.activation(inv, inv2, AF.Sqrt)

            # t1 = r * inv  (q01/q10 are folded into W2m on the host)
            t1 = prep.tile([P, NT, 3], f32)
            for c in range(3):
                nc.vector.tensor_tensor(t1[:, :, c], r3[:, :, c], inv,
                                        op=OP.mult)
            t1c = prep.tile([P, NT, 3], f32)
            nc.vector.tensor_scalar(t1c, t1, _C["w1p"], None, op0=OP.mult)

            # praw = k2 * (y*z, x*y, x*z);  t2w = praw * inv2
            praw = prep.tile([P, NT, 3], f32)
            nc.vector.scalar_tensor_tensor(
                praw[:, :, 0], r3[:, :, 1], _C["k2"], r3[:, :, 2],
                op0=OP.mult, op1=OP.mult)
            nc.vector.scalar_tensor_tensor(
                praw[:, :, 1], r3[:, :, 0], _C["k2"], r3[:, :, 1],
                op0=OP.mult, op1=OP.mult)
            nc.vector.scalar_tensor_tensor(
                praw[:, :, 2], r3[:, :, 0], _C["k2"], r3[:, :, 2],
                op0=OP.mult, op1=OP.mult)
            t2w = prep.tile([P, NT, 3], f32)
            for c in range(3):
                nc.vector.tensor_tensor(t2w[:, :, c], praw[:, :, c], inv2,
                                        op=OP.mult)

            # A2d combos
            qz2 = prep.tile([P, NT], f32)
            nc.vector.scalar_tensor_tensor(qz2, rsq[:, :, 2], 3.0, r2,
                                           op0=OP.mult, op1=OP.subtract)
            y6a = prep.tile([P, NT], f32)
            nc.vector.scalar_tensor_tensor(y6a, qz2, _C["k6"], inv2,
                                           op0=OP.mult, op1=OP.mult)
            qxmy = prep.tile([P, NT], f32)
            nc.vector.tensor_tensor(qxmy, rsq[:, :, 0], rsq[:, :, 1],
                                    op=OP.subtract)
            y8b = prep.tile([P, NT], f32)
            nc.vector.scalar_tensor_tensor(y8b, qxmy, _C["k8"], inv2,
                                           op0=OP.mult, op1=OP.mult)
            a2d = prep.tile([P, NT, 3], f32)
            nc.vector.tensor_tensor(a2d[:, :, 0], y6a, y8b, op=OP.subtract)
            nc.vector.tensor_scalar(a2d[:, :, 1], y6a, -2.0, None, op0=OP.mult)
            nc.vector.tensor_tensor(a2d[:, :, 2], y6a, y8b, op=OP.add)

            # W1/b1 broadcast to all partitions (z-layout MLP input)
            w1b = consts.tile([P, 64], f32)
            nc.sync.dma_start(
                w1b, bass.AP(tensor=w1_in, offset=0, ap=[[0, P], [1, 64]]))
            b1b = consts.tile([P, 64], f32)
            nc.sync.dma_start(
                b1b, bass.AP(tensor=b1_in, offset=0, ap=[[0, P], [1, 64]]))

            # h preact for all tiles, one batched silu
            hpre_all = prep.tile([P, NT, 64], f32)
            for t in range(NT):
                nc.vector.scalar_tensor_tensor(
                    hpre_all[:, t, :], w1b, radii[:, t:t + 1], b1b,
                    op0=OP.mult, op1=OP.add)
            h_all = prep.tile([P, NT, 64], f32)
            nc.scalar.activation(h_all, hpre_all, AF.Silu)

            # ---- per-tile loop ----
            for t in range(NT):
                hT_ps = ppsum.tile([64, P], f32, tag="hT_ps")
                nc.tensor.transpose(hT_ps, h_all[:, t, :], ident)
                hT = htp.tile([65, P], f32r)
                nc.scalar.copy(hT[0:64, :], hT_ps)
                nc.vector.tensor_copy(hT[64:65, :], ones_row)

                # V = hTaug.T @ W2m   [128, 1536] in PSUM (fp32r: 1 cyc/row)
                V = vpsum.tile([P, 1536], f32, tag="V")
                for c in range(3):
                    nc.tensor.matmul(V[:, c * 512:(c + 1) * 512], hT,
                                     w2m[:, c * 512:(c + 1) * 512])

                kt = ktp.tile([P, 4096], f32)
                ktv = kt.rearrange("p (a b) -> p a b", a=64)

                # --- ScalarE ops (read V straight from PSUM) ---
                vl2 = vsb.tile([P, 256], f32)
                nc.scalar.copy(vl2, V[:, 1280:1536])
                v01 = V[:, 256:512].rearrange("p (u v) -> p u v", u=16)
                for jm in range(3):
                    nc.scalar.mul(
                        ktv[:, 0:16, 16 + jm::3], v01,
                        t1[:, t, SIGMA[jm]:SIGMA[jm] + 1])
                v10 = V[:, 512:768].rearrange("p (u v) -> p u v", u=16)
                for im in range(3):
                    nc.scalar.mul(
                        ktv[:, 16 + im::3, 0:16], v10,
                        t1[:, t, SIGMA[im]:SIGMA[im] + 1])
                tmpd = []
                v11_1 = V[:, 1024:1280]
                for pi in range(3):
                    td = tmps.tile([P, 256], f32, tag=f"td{pi}")
                    comp = PAIR_L1_COMP[pi]
                    nc.scalar.mul(td, v11_1, t1c[:, t, comp:comp + 1])
                    tmpd.append(td)

                # --- VectorE ops ---
                nc.vector.tensor_copy(
                    ktv[:, 0:16, 0:16],
                    V[:, 0:256].rearrange("p (u v) -> p u v", u=16))
                vl2v = vl2.rearrange("p (u v) -> p u v", u=16)
                vl0sv = V[:, 768:1024].rearrange("p (u v) -> p u v", u=16)
                for i in range(3):
                    nc.vector.scalar_tensor_tensor(
                        ktv[:, 16 + i::3, 16 + i::3], vl2v,
                        a2d[:, t, i:i + 1], vl0sv,
                        op0=OP.mult, op1=OP.add)
                for pi, (a, b) in enumerate(PAIRS):
                    tdv = tmpd[pi].rearrange("p (u v) -> p u v", u=16)
                    s = PAIR_L1_SIGN[pi]
                    op_ab = OP.add if s > 0 else OP.subtract
                    op_ba = OP.subtract if s > 0 else OP.add
                    nc.vector.scalar_tensor_tensor(
                        ktv[:, 16 + a::3, 16 + b::3], vl2v,
                        t2w[:, t, pi:pi + 1], tdv,
                        op0=OP.mult, op1=op_ab)
                    nc.vector.scalar_tensor_tensor(
                        ktv[:, 16 + b::3, 16 + a::3], vl2v,
                        t2w[:, t, pi:pi + 1], tdv,
                        op0=OP.mult, op1=op_ba)

                # store: rows z = 16p + t
                out_ap = bass.AP(
                    tensor=out, offset=t * 4096,
                    ap=[[16 * 4096, P], [1, 4096]])
                nc.sync.dma_start(out_ap, kt)

    nc.compile()
    _NC_CACHE["nc"] = nc
    return nc


# ---------------------------------------------------------------------------
# host entry
# ---------------------------------------------------------------------------


def _run(inputs, trace=False):
    from concourse.bass_utils import run_bass_kernel_spmd

    r = np.ascontiguousarray(np.asarray(inputs["r"], np.float32).reshape(-1, 3))
    W1 = np.asarray(inputs["W1"], np.float32)
    b1 = np.asarray(inputs["b1"], np.float32)
    W2 = np.asarray(inputs["W2"], np.float32)
    b2 = np.asarray(inputs["b2"], np.float32)

    w2aug = np.concatenate([W2, b2[None, :]], 0)
    w2m = w2aug[:, _w2_perm()].astype(np.float64)
    # fold per-block constants into the matmul weights
    w2m[:, 0:256] *= _C["a00"]
    w2m[:, 256:512] *= _C["q01"]
    w2m[:, 512:768] *= _C["q10"]
    w2m[:, 768:1024] *= _C["w0c0"]
    w2m = np.ascontiguousarray(w2m, np.float32)
    w1c = np.ascontiguousarray(W1.reshape(64, 1))
    b1c = np.ascontiguousarray(b1.reshape(64, 1))

    nc = _build_nc()
    in_maps = []
    for c in range(N_CORES):
        in_maps.append({
            "r_in": np.ascontiguousarray(r[c * NZ:(c + 1) * NZ]),
            "w2m": w2m,
            "w1c": w1c,
            "b1c": b1c,
        })
    res = run_bass_kernel_spmd(nc, in_maps, core_ids=list(range(N_CORES)),
                               trace=trace)
    outs = [res.results[c]["out"] for c in range(N_CORES)]
    full = np.concatenate(outs, 0).reshape(16, 32, 32, 64, 64)
    return full, res


def kernel(**inputs):
    return _run(inputs, trace=False)[0]


if __name__ == "__main__":
    _build_nc()
    print("build+compile OK")
